# revision 53
# baseline (speedup 1.0000x reference)
"""CharRNNEmbedding Trainium2 kernel: 2-layer biLSTM char encoder over 8 NeuronCores.

Data-parallel: 4096 words split 512/core; weights replicated. Feature-major
activations on-chip. Layer-1 collapses to two single LSTM-cell evals (the
reference only consumes h1[0,:,:H] and h1[-1,:,H:], both first-scan-step
outputs from zero state), so w_hh_l1* and the layer-1 f-gates are unused.
Biases are folded into the matmuls via a constant-1 input row.
"""
import sys

sys.path.insert(0, "/opt/trn_rl_repo")

import numpy as np
from contextlib import ExitStack

import concourse.bass as bass
import concourse.tile as tile
import concourse.mybir as mybir
from concourse.bass_utils import run_bass_kernel_spmd

F32 = mybir.dt.float32
F32R = mybir.dt.float32r
AF = mybir.ActivationFunctionType
ALU = mybir.AluOpType

NCORES = 8
B, S, T = 32, 128, 16
VOCAB, E, H = 262, 64, 256
NC_W = B * S // NCORES          # words per core = 512
TOK = NC_W * T                  # tokens per core = 8192
CH = 256                        # scan token chunk
NCH = NC_W // CH                # chunks per step = 2
G4 = 4 * H                      # 1024

# ---- blob layout (free-dim offsets into the [128, BLOB_F] weights blob) ----
OFF_WIH0 = 0                      # [128, 2, G4]   rows 0:65 = [w_ih_l0{f,b}.T; b]
OFF_WHH0 = OFF_WIH0 + 2 * G4      # [128, 2, 2, G4] (dir, ktile)
OFF_WIH1 = OFF_WHH0 + 4 * G4      # [128, 2, 5, 768] (dir, ktile) cols=[i,o,g]
OFF_WOUT = OFF_WIH1 + 2 * 5 * 768  # [128, 5, 256]
OFF_CEMB = OFF_WOUT + 5 * 256     # [128, 3, 65]
BLOB_F = OFF_CEMB + 3 * 65


def _pack_blob(inp):
    """Host-side: pack all weights (transposed, bias-folded) into one
    [128, BLOB_F] fp32 array."""
    blob = np.zeros((128, BLOB_F), np.float32)

    def put(sec, arr):  # arr [k<=128, f]
        k, f = arr.shape
        blob[:k, sec:sec + f] = arr

    for d, nm in enumerate("fb"):
        w = np.asarray(inp[f"w_ih_l0{nm}"], np.float32)      # [1024, 64]
        b = np.asarray(inp[f"b_l0{nm}"], np.float32)         # [1024]
        aug = np.concatenate([w.T, b[None, :]], 0)           # [65, 1024]
        put(OFF_WIH0 + d * G4, aug)
        whh = np.asarray(inp[f"w_hh_l0{nm}"], np.float32).T  # [256, 1024]
        for k in range(2):
            put(OFF_WHH0 + (d * 2 + k) * G4, whh[k * 128:(k + 1) * 128])
        # layer 1: keep gates i, o, g (f-gate unused: c0 = 0)
        w1 = np.asarray(inp[f"w_ih_l1{nm}"], np.float32)     # [1024, 512]
        b1 = np.asarray(inp[f"b_l1{nm}"], np.float32)        # [1024]
        sel = np.r_[0:256, 768:1024, 512:768]                # i, o, g rows
        aug1 = np.concatenate([w1[sel].T, b1[sel][None, :]], 0)  # [513, 768]
        for k in range(5):
            put(OFF_WIH1 + (d * 5 + k) * 768, aug1[k * 128:min((k + 1) * 128, 513)])
    wo = np.asarray(inp["w_out"], np.float32)                # [256, 512]
    bo = np.asarray(inp["b_out"], np.float32)                # [256]
    aug_o = np.concatenate([wo.T, bo[None, :]], 0)           # [513, 256]
    for k in range(5):
        put(OFF_WOUT + k * 256, aug_o[k * 128:min((k + 1) * 128, 513)])
    ce = np.asarray(inp["char_emb"], np.float32)             # [262, 64]
    ce_aug = np.zeros((384, 65), np.float32)
    ce_aug[:VOCAB, :E] = ce
    ce_aug[:VOCAB, E] = 1.0
    for k in range(3):
        put(OFF_CEMB + k * 65, ce_aug[k * 128:(k + 1) * 128])
    return blob


def _legalize_waits(nc, max_waits=1):
    """This walrus build rejects >1 sync wait per instruction: split extras
    onto standalone no-ops ahead of the instruction (same engine queue)."""
    ctr = 0
    for f in nc.m.functions:
        for blk in f.blocks:
            out = []
            for inst in blk.instructions:
                si = inst.sync_info
                if si is not None and si.on_wait and len(si.on_wait) > max_waits:
                    waits = list(si.on_wait)
                    for w in waits[:-max_waits]:
                        nop = mybir.InstNoOp(name=f"I-wsplit-{ctr}")
                        ctr += 1
                        nop.engine = inst.engine
                        nop.sync_info = mybir.SyncInfo(on_wait=[w], on_update=[])
                        out.append(nop)
                    inst.sync_info = mybir.SyncInfo(
                        on_wait=waits[-max_waits:], on_update=list(si.on_update))
                out.append(inst)
            blk.instructions = out
    return nc


def build_nc(debug=False):
    nc = bass.Bass()
    # blob declared f32r: host data is fp32 bit-identical; hardware rounds on use
    blob_d = nc.dram_tensor("blob", [128, BLOB_F], F32R, kind="ExternalInput")
    ids_d = nc.dram_tensor("ids", [TOK], F32, kind="ExternalInput")   # time-major
    out_d = nc.dram_tensor("out", [128, 2, NC_W], F32, kind="ExternalOutput")
    dbg = {}
    if debug:
        for nm, shp, dt in (("dx2", [65, TOK], F32R), ("dh0", [128, 2, NC_W], F32R),
                            ("dh1", [128, 2, NC_W], F32R), ("dc0", [128, 2, NC_W], F32),
                            ("dhf", [128, 2, NC_W], F32R), ("dhb", [128, 2, NC_W], F32R),
                            ("dmerged", [128, 4, NC_W], F32R),
                            ("dgates", [128, 8, NC_W], F32)):
            dbg[nm] = nc.dram_tensor(nm, shp, dt, kind="ExternalOutput")

    with tile.TileContext(nc) as tc, ExitStack() as ctx:
        wpool = ctx.enter_context(tc.tile_pool(name="weights", bufs=1))
        spool = ctx.enter_context(tc.tile_pool(name="state", bufs=1))
        gpool = ctx.enter_context(tc.tile_pool(name="gates", bufs=2))
        lpool = ctx.enter_context(tc.tile_pool(name="l1", bufs=1))
        epool = ctx.enter_context(tc.tile_pool(name="embed", bufs=3))
        psum = ctx.enter_context(tc.tile_pool(name="ps", bufs=2, space="PSUM"))

        # ---- weights: per-section tiles; cemb+wih0 first (embedding
        # needs them), then the big hh/l1/out sections split across queues ----
        w_ce = wpool.tile([128, 3 * 65], F32R)
        nc.sync.dma_start(w_ce[:], blob_d[:, OFF_CEMB:OFF_CEMB + 3 * 65])
        w_ih0 = wpool.tile([128, 2 * G4], F32R)
        nc.sync.dma_start(w_ih0[:], blob_d[:, OFF_WIH0:OFF_WIH0 + 2 * G4])
        w_hh0 = wpool.tile([128, 4 * G4], F32R)
        nc.sync.dma_start(w_hh0[:], blob_d[:, OFF_WHH0:OFF_WHH0 + 4 * G4])
        w_ih1 = wpool.tile([128, 10 * 768], F32R)
        nc.sync.dma_start(w_ih1[:], blob_d[:, OFF_WIH1:OFF_WIH1 + 10 * 768])
        w_out = wpool.tile([128, 5 * 256], F32R)
        nc.sync.dma_start(w_out[:], blob_d[:, OFF_WOUT:OFF_WOUT + 5 * 256])

        def wih0(d):
            return w_ih0[0:65, d * G4:(d + 1) * G4]

        def whh0(d, k):
            o = (d * 2 + k) * G4
            return w_hh0[:, o:o + G4]

        def wih1(d, k):
            o = (d * 5 + k) * 768
            return w_ih1[:, o:o + 768]

        def wout(k):
            return w_out[:, k * 256:(k + 1) * 256]

        def cemb(k):
            return w_ce[:, k * 65:(k + 1) * 65]

        # iota per-partition columns: iota_c[:, k] = p + 128k
        iota_c = wpool.tile([128, 3], F32)
        for k in range(3):
            nc.gpsimd.iota(iota_c[:, k:k + 1], pattern=[[0, 1]], base=128 * k,
                           channel_multiplier=1,
                           allow_small_or_imprecise_dtypes=True)

        # ---- embedding: onehot matmul -> x2 [65, TOK] (row 64 = 1.0) ----
        x2 = spool.tile([65, TOK], F32R)
        for t in range(T):
            idsB = epool.tile([128, NC_W], F32, tag="idsB")
            bc = bass.AP(tensor=ids_d[:].tensor, offset=t * NC_W,
                         ap=[[0, 128], [1, NC_W]])
            nc.scalar.dma_start(idsB[:], bc)
            ps_x = psum.tile([65, NC_W], F32, tag="ps")
            for k in range(3):
                oh = epool.tile([128, NC_W], F32R, tag="oh")
                eng = nc.vector if k == 0 else nc.gpsimd
                eng.tensor_scalar(oh[:], idsB[:], iota_c[:, k:k + 1], None,
                                  op0=ALU.is_equal)
                nc.tensor.matmul(ps_x[:], cemb(k), oh[:],
                                 start=(k == 0), stop=(k == 2))
            nc.vector.tensor_copy(x2[:, t * NC_W:(t + 1) * NC_W], ps_x[:])

        # ---- layer-0 biLSTM scan ----
        h = {d: spool.tile([128, 2, NC_W], F32R, tag=f"h{d}", name=f"h{d}")
             for d in range(2)}
        c = {d: spool.tile([128, 2, NC_W], F32, tag=f"c{d}", name=f"c{d}")
             for d in range(2)}
        # snapshots for layer 1 (the other end's state comes from h itself)
        h0_f0 = spool.tile([128, 2, NC_W], F32R)   # h_fwd after t=0
        hb_15 = spool.tile([128, 2, NC_W], F32R)   # h_bwd after its first step
        ones = spool.tile([128, 1, NC_W], F32R)
        ones_f = epool.tile([128, 1, NC_W], F32, tag="idsB")
        nc.vector.memset(ones_f[:], 0.0)
        nc.vector.memset(ones_f[0:1, :, :], 1.0)
        nc.vector.tensor_copy(ones[:], ones_f[:])

        pending = []   # deferred (tanh_c, h-mul) tails, one unit deep

        def flush_tail():
            # split into k-halves: h[:,0,:] lands ~1us earlier, unblocking
            # the next same-dir unit's k0 hproj matmuls
            if not pending:
                return
            pt, pd, psigo = pending.pop()
            for k in range(2):
                tc_ = gpool.tile([128, 1, NC_W], F32, tag=f"tc{k}",
                                 name=f"tc{k}_{pt}_{pd}")
                nc.scalar.activation(tc_[:], c[pd][:, k:k + 1, :], AF.Tanh)
                nc.vector.tensor_mul(h[pd][:, k:k + 1, :], psigo[:, k:k + 1, :],
                                     tc_[:])

        def scan_dir(t, d):
            """One (step, dir) unit in two PSUM waves of 4 full banks each:
            wave A = gates [i0 i1 f0 f1], wave B = [o0 o1 g0 g1]. Each
            accumulation group owns a full [128,512] bank (a start=True
            matmul clears its whole bank). bufs=2 on the psum pool lets PE
            fill one wave while ACT drains the other. The (tanh_c, h-mul)
            tail is deferred one unit so it doesn't head-of-line-block the
            next unit's sigA on the ACT queue."""
            xt = t if d == 0 else (T - 1 - t)
            xcols = slice(xt * NC_W, (xt + 1) * NC_W)

            def wave(mtiles, name):
                gp = psum.tile([128, 4, NC_W], F32, tag="ps", name=name)
                for pos, m in enumerate(mtiles):
                    nc.tensor.matmul(gp[:, pos, :],
                                     wih0(d)[:, m * 128:(m + 1) * 128],
                                     x2[:, xcols], start=True, stop=(t == 0))
                if t > 0:
                    for k in range(2):
                        for pos, m in enumerate(mtiles):
                            nc.tensor.matmul(gp[:, pos, :],
                                             whh0(d, k)[:, m * 128:(m + 1) * 128],
                                             h[d][:, k, :],
                                             start=False, stop=(k == 1))
                return gp

            gpA = wave([0, 1, 2, 3], f"gpA_{t}_{d}")      # i, f
            sigA = gpool.tile([128, 4, NC_W], F32, tag="sigA")
            nc.scalar.activation(sigA[:], gpA[:], AF.Sigmoid)
            flush_tail()   # other dir's (tanh_c, h-mul): early on ACT queue
            if t > 0:
                for k in range(2):
                    nc.vector.tensor_mul(c[d][:, k:k + 1, :], c[d][:, k:k + 1, :],
                                         sigA[:, 2 + k:3 + k, :])
            gpB = wave([6, 7, 4, 5], f"gpB_{t}_{d}")      # o, g
            if debug and t == 1 and d == 0:
                nc.sync.dma_start(dbg["dgates"][:, 0:4, :], gpA[:])
                nc.sync.dma_start(dbg["dgates"][:, 4:8, :], gpB[:])
            sigo = gpool.tile([128, 2, NC_W], F32, tag="sigo", bufs=3)
            nc.scalar.activation(sigo[:], gpB[:, 0:2, :], AF.Sigmoid)
            tg = gpool.tile([128, 2, NC_W], F32, tag="tg", bufs=3)
            nc.scalar.activation(tg[:], gpB[:, 2:4, :], AF.Tanh)
            t1 = gpool.tile([128, 2, NC_W], F32, tag="t1")
            for k in range(2):  # k-split so c(k0) finishes earlier
                kk = slice(k, k + 1)
                nc.vector.tensor_mul(t1[:, kk, :], sigA[:, k:k + 1, :], tg[:, kk, :])
                if t == 0:
                    nc.vector.tensor_copy(c[d][:, kk, :], t1[:, kk, :])
                else:
                    nc.vector.tensor_add(c[d][:, kk, :], c[d][:, kk, :],
                                         t1[:, kk, :])
            pending.append((t, d, sigo))

        for t in range(T):
            for d in range(2):
                scan_dir(t, d)
            if t == 0:
                flush_tail()        # h_f(0) and h_b(first) complete
                nc.vector.tensor_copy(h0_f0[:], h[0][:])
                nc.vector.tensor_copy(hb_15[:], h[1][:])
                if debug:
                    nc.sync.dma_start(dbg["dh0"][:], h[0][:])
                    nc.sync.dma_start(dbg["dh1"][:], h[1][:])
                    nc.sync.dma_start(dbg["dc0"][:], c[0][:])

        flush_tail()
        # ---- layer 1 (two single cells) + output projection ----
        # fwd cell input h0[t=0]  = [h0_f0 ; h_b(final) ; ones]
        # bwd cell input h0[t=15] = [h_f(final) ; hb_15 ; ones]
        merged = lpool.tile([128, 4, NC_W], F32R)
        l1_rhs = {
            0: [h0_f0[:, 0, :], h0_f0[:, 1, :], h[1][:, 0, :], h[1][:, 1, :],
                ones[:, 0, :]],
            1: [h[0][:, 0, :], h[0][:, 1, :], hb_15[:, 0, :], hb_15[:, 1, :],
                ones[:, 0, :]],
        }
        for d in range(2):
            # wave tiles: g1A = [i0 i1 g0 g1], g1B = [o0 o1]; W col-tiles
            # in wih1 are packed [i(0,1) o(2,3) g(4,5)]
            g1A = psum.tile([128, 4, NC_W], F32, tag="ps", name=f"g1A_{d}")
            g1B = psum.tile([128, 2, NC_W], F32, tag="ps", name=f"g1B_{d}")
            KORD = [4, 0, 1, 2, 3]   # ones first, early h0, scan-final h last
            for pos, m in enumerate([0, 1, 4, 5]):
                for j, k in enumerate(KORD):
                    nc.tensor.matmul(g1A[:, pos, :],
                                     wih1(d, k)[:, m * 128:(m + 1) * 128],
                                     l1_rhs[d][k], start=(j == 0), stop=(j == 4))
            for pos, m in enumerate([2, 3]):
                for j, k in enumerate(KORD):
                    nc.tensor.matmul(g1B[:, pos, :],
                                     wih1(d, k)[:, m * 128:(m + 1) * 128],
                                     l1_rhs[d][k], start=(j == 0), stop=(j == 4))
            sig1 = gpool.tile([128, 2, NC_W], F32, tag="sigo", bufs=3)
            nc.scalar.activation(sig1[:], g1A[:, 0:2, :], AF.Sigmoid)
            tg1 = gpool.tile([128, 2, NC_W], F32, tag="tg", bufs=3)
            nc.scalar.activation(tg1[:], g1A[:, 2:4, :], AF.Tanh)
            so1 = gpool.tile([128, 2, NC_W], F32, tag="sigA")
            nc.scalar.activation(so1[:], g1B[:], AF.Sigmoid)
            for k in range(2):   # k-split: merged(k0) unchains po matmuls
                kk = slice(k, k + 1)
                c1 = gpool.tile([128, 1, NC_W], F32, tag="t1", name=f"c1_{d}_{k}")
                nc.vector.tensor_mul(c1[:], sig1[:, kk, :], tg1[:, kk, :])
                tc1 = gpool.tile([128, 1, NC_W], F32, tag="tc0", name=f"tc1_{d}_{k}")
                nc.scalar.activation(tc1[:], c1[:], AF.Tanh)
                nc.vector.tensor_mul(merged[:, d * 2 + k:d * 2 + k + 1, :],
                                     so1[:, kk, :], tc1[:])

        ob = lpool.tile([128, 2, NC_W], F32)
        po = psum.tile([128, 2, NC_W], F32, tag="ps")
        mr = [merged[:, 0, :], merged[:, 1, :],
              merged[:, 2, :], merged[:, 3, :], ones[:, 0, :]]
        for m in range(2):
            for j, k in enumerate([4, 0, 1, 2, 3]):
                nc.tensor.matmul(po[:, m, :], wout(k)[:, m * 128:(m + 1) * 128],
                                 mr[k], start=(j == 0), stop=(j == 4))
        nc.scalar.copy(ob[:], po[:])
        nc.sync.dma_start(out_d[:], ob[:])

    _legalize_waits(nc)
    return nc


_NC_CACHE = None


def kernel(**inputs):
    global _NC_CACHE
    if _NC_CACHE is None:
        _NC_CACHE = build_nc()
    nc = _NC_CACHE

    blob = _pack_blob(inputs)
    char_ids = np.asarray(inputs["char_ids"])
    in_maps = []
    for cc in range(NCORES):
        ids_c = char_ids.reshape(B * S, T)[cc * NC_W:(cc + 1) * NC_W]   # [512, 16]
        ids_tm = np.ascontiguousarray(ids_c.T).astype(np.float32).reshape(TOK)
        in_maps.append({"blob": blob, "ids": ids_tm})

    res = run_bass_kernel_spmd(nc, in_maps, list(range(NCORES)))

    outs = []
    for cc in range(NCORES):
        o = res.results[cc]["out"]                 # [128, 2, 512]: feat = m*128+p
        outs.append(o.transpose(1, 0, 2).reshape(256, NC_W).T)   # [512, 256]
    full = np.concatenate(outs, 0)                 # [4096, 256]
    return full.reshape(B, S, H).astype(np.float32)


# revision 54
# speedup vs baseline: 1.0018x; 1.0018x over previous
"""CharRNNEmbedding Trainium2 kernel: 2-layer biLSTM char encoder over 8 NeuronCores.

Data-parallel: 4096 words split 512/core; weights replicated. Feature-major
activations on-chip. Layer-1 collapses to two single LSTM-cell evals (the
reference only consumes h1[0,:,:H] and h1[-1,:,H:], both first-scan-step
outputs from zero state), so w_hh_l1* and the layer-1 f-gates are unused.
Biases are folded into the matmuls via a constant-1 input row.
"""
import sys

sys.path.insert(0, "/opt/trn_rl_repo")

import numpy as np
from contextlib import ExitStack

import concourse.bass as bass
import concourse.tile as tile
import concourse.mybir as mybir
from concourse.bass_utils import run_bass_kernel_spmd

F32 = mybir.dt.float32
F32R = mybir.dt.float32r
AF = mybir.ActivationFunctionType
ALU = mybir.AluOpType

NCORES = 8
B, S, T = 32, 128, 16
VOCAB, E, H = 262, 64, 256
NC_W = B * S // NCORES          # words per core = 512
TOK = NC_W * T                  # tokens per core = 8192
CH = 256                        # scan token chunk
NCH = NC_W // CH                # chunks per step = 2
G4 = 4 * H                      # 1024

# ---- blob layout (free-dim offsets into the [128, BLOB_F] weights blob) ----
OFF_WIH0 = 0                      # [128, 2, G4]   rows 0:65 = [w_ih_l0{f,b}.T; b]
OFF_WHH0 = OFF_WIH0 + 2 * G4      # [128, 2, 2, G4] (dir, ktile)
OFF_WIH1 = OFF_WHH0 + 4 * G4      # [128, 2, 5, 768] (dir, ktile) cols=[i,o,g]
OFF_WOUT = OFF_WIH1 + 2 * 5 * 768  # [128, 5, 256]
OFF_CEMB = OFF_WOUT + 5 * 256     # [128, 3, 65]
BLOB_F = OFF_CEMB + 3 * 65


def _pack_blob(inp):
    """Host-side: pack all weights (transposed, bias-folded) into one
    [128, BLOB_F] fp32 array."""
    blob = np.zeros((128, BLOB_F), np.float32)

    def put(sec, arr):  # arr [k<=128, f]
        k, f = arr.shape
        blob[:k, sec:sec + f] = arr

    for d, nm in enumerate("fb"):
        w = np.asarray(inp[f"w_ih_l0{nm}"], np.float32)      # [1024, 64]
        b = np.asarray(inp[f"b_l0{nm}"], np.float32)         # [1024]
        aug = np.concatenate([w.T, b[None, :]], 0)           # [65, 1024]
        put(OFF_WIH0 + d * G4, aug)
        whh = np.asarray(inp[f"w_hh_l0{nm}"], np.float32).T  # [256, 1024]
        for k in range(2):
            put(OFF_WHH0 + (d * 2 + k) * G4, whh[k * 128:(k + 1) * 128])
        # layer 1: keep gates i, o, g (f-gate unused: c0 = 0)
        w1 = np.asarray(inp[f"w_ih_l1{nm}"], np.float32)     # [1024, 512]
        b1 = np.asarray(inp[f"b_l1{nm}"], np.float32)        # [1024]
        sel = np.r_[0:256, 768:1024, 512:768]                # i, o, g rows
        aug1 = np.concatenate([w1[sel].T, b1[sel][None, :]], 0)  # [513, 768]
        for k in range(5):
            put(OFF_WIH1 + (d * 5 + k) * 768, aug1[k * 128:min((k + 1) * 128, 513)])
    wo = np.asarray(inp["w_out"], np.float32)                # [256, 512]
    bo = np.asarray(inp["b_out"], np.float32)                # [256]
    aug_o = np.concatenate([wo.T, bo[None, :]], 0)           # [513, 256]
    for k in range(5):
        put(OFF_WOUT + k * 256, aug_o[k * 128:min((k + 1) * 128, 513)])
    ce = np.asarray(inp["char_emb"], np.float32)             # [262, 64]
    ce_aug = np.zeros((384, 65), np.float32)
    ce_aug[:VOCAB, :E] = ce
    ce_aug[:VOCAB, E] = 1.0
    for k in range(3):
        put(OFF_CEMB + k * 65, ce_aug[k * 128:(k + 1) * 128])
    return blob


def _legalize_waits(nc, max_waits=1):
    """This walrus build rejects >1 sync wait per instruction: split extras
    onto standalone no-ops ahead of the instruction (same engine queue)."""
    ctr = 0
    for f in nc.m.functions:
        for blk in f.blocks:
            out = []
            for inst in blk.instructions:
                si = inst.sync_info
                if si is not None and si.on_wait and len(si.on_wait) > max_waits:
                    waits = list(si.on_wait)
                    for w in waits[:-max_waits]:
                        nop = mybir.InstNoOp(name=f"I-wsplit-{ctr}")
                        ctr += 1
                        nop.engine = inst.engine
                        nop.sync_info = mybir.SyncInfo(on_wait=[w], on_update=[])
                        out.append(nop)
                    inst.sync_info = mybir.SyncInfo(
                        on_wait=waits[-max_waits:], on_update=list(si.on_update))
                out.append(inst)
            blk.instructions = out
    return nc


def build_nc(debug=False):
    nc = bass.Bass()
    # blob declared f32r: host data is fp32 bit-identical; hardware rounds on use
    blob_d = nc.dram_tensor("blob", [128, BLOB_F], F32R, kind="ExternalInput")
    ids_d = nc.dram_tensor("ids", [TOK], F32, kind="ExternalInput")   # time-major
    out_d = nc.dram_tensor("out", [128, 2, NC_W], F32, kind="ExternalOutput")
    dbg = {}
    if debug:
        for nm, shp, dt in (("dx2", [65, TOK], F32R), ("dh0", [128, 2, NC_W], F32R),
                            ("dh1", [128, 2, NC_W], F32R), ("dc0", [128, 2, NC_W], F32),
                            ("dhf", [128, 2, NC_W], F32R), ("dhb", [128, 2, NC_W], F32R),
                            ("dmerged", [128, 4, NC_W], F32R),
                            ("dgates", [128, 8, NC_W], F32)):
            dbg[nm] = nc.dram_tensor(nm, shp, dt, kind="ExternalOutput")

    with tile.TileContext(nc) as tc, ExitStack() as ctx:
        wpool = ctx.enter_context(tc.tile_pool(name="weights", bufs=1))
        spool = ctx.enter_context(tc.tile_pool(name="state", bufs=1))
        gpool = ctx.enter_context(tc.tile_pool(name="gates", bufs=2))
        lpool = ctx.enter_context(tc.tile_pool(name="l1", bufs=1))
        epool = ctx.enter_context(tc.tile_pool(name="embed", bufs=3))
        psum = ctx.enter_context(tc.tile_pool(name="ps", bufs=2, space="PSUM"))

        # ---- weights: per-section tiles; cemb+wih0 first (embedding
        # needs them), then the big hh/l1/out sections split across queues ----
        w_ce = wpool.tile([128, 3 * 65], F32R)
        nc.sync.dma_start(w_ce[:], blob_d[:, OFF_CEMB:OFF_CEMB + 3 * 65])
        w_ih0 = wpool.tile([128, 2 * G4], F32R)
        nc.sync.dma_start(w_ih0[:], blob_d[:, OFF_WIH0:OFF_WIH0 + 2 * G4])
        w_hh0 = wpool.tile([128, 4 * G4], F32R)
        nc.sync.dma_start(w_hh0[:], blob_d[:, OFF_WHH0:OFF_WHH0 + 4 * G4])
        w_ih1 = wpool.tile([128, 10 * 768], F32R)
        nc.sync.dma_start(w_ih1[:], blob_d[:, OFF_WIH1:OFF_WIH1 + 10 * 768])
        w_out = wpool.tile([128, 5 * 256], F32R)
        nc.sync.dma_start(w_out[:], blob_d[:, OFF_WOUT:OFF_WOUT + 5 * 256])

        def wih0(d):
            return w_ih0[0:65, d * G4:(d + 1) * G4]

        def whh0(d, k):
            o = (d * 2 + k) * G4
            return w_hh0[:, o:o + G4]

        def wih1(d, k):
            o = (d * 5 + k) * 768
            return w_ih1[:, o:o + 768]

        def wout(k):
            return w_out[:, k * 256:(k + 1) * 256]

        def cemb(k):
            return w_ce[:, k * 65:(k + 1) * 65]

        # iota per-partition columns: iota_c[:, k] = p + 128k
        iota_c = wpool.tile([128, 3], F32)
        for k in range(3):
            nc.gpsimd.iota(iota_c[:, k:k + 1], pattern=[[0, 1]], base=128 * k,
                           channel_multiplier=1,
                           allow_small_or_imprecise_dtypes=True)

        # ---- embedding: onehot matmul -> x2 [65, TOK] (row 64 = 1.0) ----
        x2 = spool.tile([65, TOK], F32R)
        for t in range(T):
            idsB = epool.tile([128, NC_W], F32, tag="idsB")
            bc = bass.AP(tensor=ids_d[:].tensor, offset=t * NC_W,
                         ap=[[0, 128], [1, NC_W]])
            nc.scalar.dma_start(idsB[:], bc)
            ps_x = psum.tile([65, NC_W], F32, tag="ps")
            for k in range(3):
                oh = epool.tile([128, NC_W], F32R, tag="oh")
                eng = nc.vector if k == 0 else nc.gpsimd
                eng.tensor_scalar(oh[:], idsB[:], iota_c[:, k:k + 1], None,
                                  op0=ALU.is_equal)
                nc.tensor.matmul(ps_x[:], cemb(k), oh[:],
                                 start=(k == 0), stop=(k == 2))
            nc.vector.tensor_copy(x2[:, t * NC_W:(t + 1) * NC_W], ps_x[:])

        # ---- layer-0 biLSTM scan ----
        h = {d: spool.tile([128, 2, NC_W], F32R, tag=f"h{d}", name=f"h{d}")
             for d in range(2)}
        c = {d: spool.tile([128, 2, NC_W], F32, tag=f"c{d}", name=f"c{d}")
             for d in range(2)}
        # snapshots for layer 1 (the other end's state comes from h itself)
        h0_f0 = spool.tile([128, 2, NC_W], F32R)   # h_fwd after t=0
        hb_15 = spool.tile([128, 2, NC_W], F32R)   # h_bwd after its first step
        ones = spool.tile([128, 1, NC_W], F32R)
        ones_f = epool.tile([128, 1, NC_W], F32, tag="idsB")
        nc.vector.memset(ones_f[:], 0.0)
        nc.vector.memset(ones_f[0:1, :, :], 1.0)
        nc.vector.tensor_copy(ones[:], ones_f[:])

        pending = []   # deferred (tanh_c, h-mul) tails, one unit deep

        def flush_tail():
            # split into k-halves: h[:,0,:] lands ~1us earlier, unblocking
            # the next same-dir unit's k0 hproj matmuls
            if not pending:
                return
            pt, pd, psigo = pending.pop()
            for k in range(2):
                tc_ = gpool.tile([128, 1, NC_W], F32, tag=f"tc{k}",
                                 name=f"tc{k}_{pt}_{pd}")
                nc.scalar.activation(tc_[:], c[pd][:, k:k + 1, :], AF.Tanh)
                nc.vector.tensor_mul(h[pd][:, k:k + 1, :], psigo[:, k:k + 1, :],
                                     tc_[:])

        def scan_dir(t, d):
            """One (step, dir) unit in two PSUM waves of 4 full banks each:
            wave A = gates [i0 i1 f0 f1], wave B = [o0 o1 g0 g1]. Each
            accumulation group owns a full [128,512] bank (a start=True
            matmul clears its whole bank). bufs=2 on the psum pool lets PE
            fill one wave while ACT drains the other. The (tanh_c, h-mul)
            tail is deferred one unit so it doesn't head-of-line-block the
            next unit's sigA on the ACT queue."""
            xt = t if d == 0 else (T - 1 - t)
            xcols = slice(xt * NC_W, (xt + 1) * NC_W)

            def wave(mtiles, name):
                gp = psum.tile([128, 4, NC_W], F32, tag="ps", name=name)
                for pos, m in enumerate(mtiles):
                    nc.tensor.matmul(gp[:, pos, :],
                                     wih0(d)[:, m * 128:(m + 1) * 128],
                                     x2[:, xcols], start=True, stop=(t == 0))
                if t > 0:
                    for k in range(2):
                        for pos, m in enumerate(mtiles):
                            nc.tensor.matmul(gp[:, pos, :],
                                             whh0(d, k)[:, m * 128:(m + 1) * 128],
                                             h[d][:, k, :],
                                             start=False, stop=(k == 1))
                return gp

            gpA = wave([0, 1, 2, 3], f"gpA_{t}_{d}")      # i, f
            sigA = gpool.tile([128, 4, NC_W], F32, tag="sigA")
            nc.scalar.activation(sigA[:], gpA[:], AF.Sigmoid)
            flush_tail()   # other dir's (tanh_c, h-mul): early on ACT queue
            if t > 0:
                for k in range(2):
                    nc.vector.tensor_mul(c[d][:, k:k + 1, :], c[d][:, k:k + 1, :],
                                         sigA[:, 2 + k:3 + k, :])
            gpB = wave([6, 7, 4, 5], f"gpB_{t}_{d}")      # o, g
            if debug and t == 1 and d == 0:
                nc.sync.dma_start(dbg["dgates"][:, 0:4, :], gpA[:])
                nc.sync.dma_start(dbg["dgates"][:, 4:8, :], gpB[:])
            sigo = gpool.tile([128, 2, NC_W], F32, tag="sigo", bufs=3)
            nc.scalar.activation(sigo[:], gpB[:, 0:2, :], AF.Sigmoid)
            tg = gpool.tile([128, 2, NC_W], F32, tag="tg", bufs=3)
            nc.scalar.activation(tg[:], gpB[:, 2:4, :], AF.Tanh)
            t1 = gpool.tile([128, 2, NC_W], F32, tag="t1")
            for k in range(2):  # k-split so c(k0) finishes earlier
                kk = slice(k, k + 1)
                nc.vector.tensor_mul(t1[:, kk, :], sigA[:, k:k + 1, :], tg[:, kk, :])
                if t == 0:
                    nc.vector.tensor_copy(c[d][:, kk, :], t1[:, kk, :])
                else:
                    nc.vector.tensor_add(c[d][:, kk, :], c[d][:, kk, :],
                                         t1[:, kk, :])
            pending.append((t, d, sigo))

        for t in range(T):
            for d in range(2):
                scan_dir(t, d)
            if t == 0:
                flush_tail()        # h_f(0) and h_b(first) complete
                nc.vector.tensor_copy(h0_f0[:], h[0][:])
                nc.vector.tensor_copy(hb_15[:], h[1][:])
                if debug:
                    nc.sync.dma_start(dbg["dh0"][:], h[0][:])
                    nc.sync.dma_start(dbg["dh1"][:], h[1][:])
                    nc.sync.dma_start(dbg["dc0"][:], c[0][:])

        flush_tail()
        # ---- layer 1 (two single cells) + output projection ----
        # fwd cell input h0[t=0]  = [h0_f0 ; h_b(final) ; ones]
        # bwd cell input h0[t=15] = [h_f(final) ; hb_15 ; ones]
        merged = lpool.tile([128, 4, NC_W], F32R)
        l1_rhs = {
            0: [h0_f0[:, 0, :], h0_f0[:, 1, :], h[1][:, 0, :], h[1][:, 1, :],
                ones[:, 0, :]],
            1: [h[0][:, 0, :], h[0][:, 1, :], hb_15[:, 0, :], hb_15[:, 1, :],
                ones[:, 0, :]],
        }
        for d in range(2):
            # wave tiles: g1A = [i0 i1 g0 g1], g1B = [o0 o1]; W col-tiles
            # in wih1 are packed [i(0,1) o(2,3) g(4,5)]
            g1A = psum.tile([128, 4, NC_W], F32, tag="ps", name=f"g1A_{d}")
            g1B = psum.tile([128, 2, NC_W], F32, tag="ps", name=f"g1B_{d}")
            KORD = [4, 0, 1, 2, 3]   # ones first, early h0, scan-final h last
            for pos, m in enumerate([0, 1, 4, 5]):
                for j, k in enumerate(KORD):
                    nc.tensor.matmul(g1A[:, pos, :],
                                     wih1(d, k)[:, m * 128:(m + 1) * 128],
                                     l1_rhs[d][k], start=(j == 0), stop=(j == 4))
            for pos, m in enumerate([2, 3]):
                for j, k in enumerate(KORD):
                    nc.tensor.matmul(g1B[:, pos, :],
                                     wih1(d, k)[:, m * 128:(m + 1) * 128],
                                     l1_rhs[d][k], start=(j == 0), stop=(j == 4))
            sig1 = gpool.tile([128, 2, NC_W], F32, tag="sigo", bufs=3)
            nc.scalar.activation(sig1[:], g1A[:, 0:2, :], AF.Sigmoid)
            tg1 = gpool.tile([128, 2, NC_W], F32, tag="tg", bufs=3)
            nc.scalar.activation(tg1[:], g1A[:, 2:4, :], AF.Tanh)
            so1 = gpool.tile([128, 2, NC_W], F32, tag="sigA")
            nc.scalar.activation(so1[:], g1B[:], AF.Sigmoid)
            for k in range(2):   # k-split: merged(k0) unchains po matmuls
                kk = slice(k, k + 1)
                c1 = gpool.tile([128, 1, NC_W], F32, tag="t1", name=f"c1_{d}_{k}")
                nc.vector.tensor_mul(c1[:], sig1[:, kk, :], tg1[:, kk, :])
                tc1 = gpool.tile([128, 1, NC_W], F32, tag="tc0", name=f"tc1_{d}_{k}")
                nc.scalar.activation(tc1[:], c1[:], AF.Tanh)
                nc.vector.tensor_mul(merged[:, d * 2 + k:d * 2 + k + 1, :],
                                     so1[:, kk, :], tc1[:])

        ob = lpool.tile([128, 2, NC_W], F32)
        po = psum.tile([128, 2, NC_W], F32, tag="ps")
        mr = [merged[:, 0, :], merged[:, 1, :],
              merged[:, 2, :], merged[:, 3, :], ones[:, 0, :]]
        for m in range(2):   # per-Mtile copy+DMA overlaps the other tile's MMs
            for j, k in enumerate([4, 0, 1, 2, 3]):
                nc.tensor.matmul(po[:, m, :], wout(k)[:, m * 128:(m + 1) * 128],
                                 mr[k], start=(j == 0), stop=(j == 4))
            nc.scalar.copy(ob[:, m, :], po[:, m, :])
            nc.sync.dma_start(out_d[:, m, :], ob[:, m, :])

    _legalize_waits(nc)
    return nc


_NC_CACHE = None


def kernel(**inputs):
    global _NC_CACHE
    if _NC_CACHE is None:
        _NC_CACHE = build_nc()
    nc = _NC_CACHE

    blob = _pack_blob(inputs)
    char_ids = np.asarray(inputs["char_ids"])
    in_maps = []
    for cc in range(NCORES):
        ids_c = char_ids.reshape(B * S, T)[cc * NC_W:(cc + 1) * NC_W]   # [512, 16]
        ids_tm = np.ascontiguousarray(ids_c.T).astype(np.float32).reshape(TOK)
        in_maps.append({"blob": blob, "ids": ids_tm})

    res = run_bass_kernel_spmd(nc, in_maps, list(range(NCORES)))

    outs = []
    for cc in range(NCORES):
        o = res.results[cc]["out"]                 # [128, 2, 512]: feat = m*128+p
        outs.append(o.transpose(1, 0, 2).reshape(256, NC_W).T)   # [512, 256]
    full = np.concatenate(outs, 0)                 # [4096, 256]
    return full.reshape(B, S, H).astype(np.float32)
